# revision 11
# baseline (speedup 1.0000x reference)
"""Trainium2 Bass kernel for a 2-layer GCN (GCNConv -> ReLU -> GCNConv -> ReLU ->
mean-pool -> FC) on a 100k-node / 3.2M-edge graph, SPMD across 8 NeuronCores.

Sharding: destination sharding. Edges (plus self-loops) are sorted by
destination; core c owns destinations [c*12500, (c+1)*12500). Consecutive
destinations are greedily packed into "windows" of at most 128 destinations
and 4 tiles of 128 edge slots; tile k of a window holds the window's edges
whose source row lives in table chunk k (the global padded table is split in
4 chunks of <= 32768 rows so dma_gather's int16 indices reach every row).
Per window the kernel:
  - gathers 4x128 source-node table rows (256-byte rows, bf16) with batched
    `dma_gather` calls (one per chunk per NWG windows),
  - builds a one-hot selection matrix M[e, d] = (rel[e] == d) on the DVE
    (iota vs per-slot rel compare),
  - accumulates psum[d, f] = sum_e M[e,d] * G[e,f] with 4 PE matmuls into
    one PSUM bank, optionally adds the bias as a diag(u) matmul,
  - applies relu(dinv[d] * psum) with one ACT instruction (per-partition
    scale), and
  - scatters the finished rows back to HBM with `dma_scatter_add` into a
    pre-zeroed buffer (layer 1) or accumulates the global feature sum with a
    ones-matmul (layer 2).
Node tables are stored pre-scaled by dinv[src], so edge contributions are
plain sums; self-loops are ordinary edges. Tables are exchanged between
layers with an AllGather; the final [2]-vector FC runs on-device after an
AllReduce of the feature sums.
"""

import numpy as np
import ml_dtypes

import concourse.bass as bass
import concourse.bacc as bacc
import concourse.mybir as mybir
import concourse.tile as tile
import concourse.bass_utils as bass_utils

P = 128
N_CORES = 8
NCHUNK = 4              # table chunks (int16 index reach)

STAGE_DT = mybir.dt.bfloat16
STAGE_NP = ml_dtypes.bfloat16

TPW = NCHUNK            # tiles per window (one per chunk)
SLOTS = TPW * P         # edge slots per window
NWG = 16                # windows per gather batch
NWF = 16                # windows per flush-scatter batch


def _wrap16(flat):
    """dma_gather/scatter_add index layout: idx i -> partition i%16,
    col i//16, replicated across the eight 16-partition groups."""
    n = flat.size
    assert n % 16 == 0
    arr = np.ascontiguousarray(flat.reshape(n // 16, 16).T)
    return np.tile(arr, (8, 1)).astype(np.int16)


# ----------------------------------------------------------------- host prep

def prep(edge_index, n_nodes, n_cores):
    """Build per-core index tensors. Pure index manipulation (no float math
    on the graph signal)."""
    dsh = n_nodes // n_cores
    assert dsh * n_cores == n_nodes
    dsh_pad = ((dsh + P - 1) // P) * P
    ntab = n_cores * dsh_pad
    assert ntab % NCHUNK == 0
    csz = ntab // NCHUNK
    assert csz <= 32767

    src = edge_index[0].astype(np.int64)
    dst = edge_index[1].astype(np.int64)
    deg = np.bincount(dst, minlength=n_nodes).astype(np.int64) + 1  # + self loop

    loops = np.arange(n_nodes, dtype=np.int64)
    src_a = np.concatenate([src, loops])
    dst_a = np.concatenate([dst, loops])
    order = np.argsort(dst_a, kind="stable")
    src_g = src_a[order]
    dst_g = dst_a[order]
    # global row id in the AllGather-concatenated (padded) table layout
    gid_g = (src_g // dsh) * dsh_pad + (src_g % dsh)

    pre = []
    for c in range(n_cores):
        lo, hi = c * dsh, (c + 1) * dsh
        a = np.searchsorted(dst_g, lo)
        b = np.searchsorted(dst_g, hi)
        gid = gid_g[a:b]
        d = dst_g[a:b] - lo
        ck = gid // csz
        # per (dest, chunk) counts
        cnt = np.bincount(d * NCHUNK + ck, minlength=dsh * NCHUNK).reshape(dsh, NCHUNK)
        # greedy window packing: <= P dests, <= P edges per chunk
        wins = []
        d0 = 0
        run = np.zeros(NCHUNK, np.int64)
        while d0 < dsh:
            run[:] = 0
            dd = d0
            while dd < dsh and (dd - d0) < P and np.all(run + cnt[dd] <= P):
                run += cnt[dd]
                dd += 1
            if dd == d0:
                raise ValueError(f"dest {d0} chunk degree {cnt[d0].max()} > {P}")
            wins.append((d0, dd))
            d0 = dd
        pre.append((wins, gid, d, ck))

    W = max(len(p[0]) for p in pre)
    W = ((W + max(NWG, NWF) - 1) // max(NWG, NWF)) * max(NWG, NWF)

    out = []
    for c in range(n_cores):
        wins, gid, d, ck = pre[c]
        bounds = np.searchsorted(d, np.arange(dsh + 1))
        idx16 = np.zeros((W, NCHUNK, P), np.int16)       # pad -> row 0 of chunk
        rel = np.full((W, NCHUNK, P), -1.0, np.float32)  # pad -> -1
        destid = np.zeros((W, P), np.int64)
        deg_winT = np.ones((W, P), np.float32)
        deg_winT0 = np.zeros((W, P), np.float32)
        for w, (d0, d1) in enumerate(wins):
            e0, e1 = bounds[d0], bounds[d1]
            gw, dw, cw = gid[e0:e1], d[e0:e1], ck[e0:e1]
            for k in range(NCHUNK):
                m = cw == k
                n = int(m.sum())
                idx16[w, k, :n] = (gw[m] - k * csz).astype(np.int16)
                rel[w, k, :n] = (dw[m] - d0).astype(np.float32)
            nd = d1 - d0
            destid[w, :nd] = d0 + np.arange(nd)
            destid[w, nd:] = dsh_pad + (np.arange(P - nd) % P)
            dg = deg[c * dsh + d0: c * dsh + d1].astype(np.float32)
            deg_winT[w, :nd] = dg
            deg_winT0[w, :nd] = dg
        for w in range(len(wins), W):
            destid[w, :] = dsh_pad + (np.arange(P) % P)

        # device layouts -----------------------------------------------------
        # rel: slot p of tile k of window w -> [p, w*TPW + k]
        rel_dev = np.ascontiguousarray(
            rel.transpose(2, 0, 1).reshape(P, W * TPW)).astype(STAGE_NP)
        # gather idx: group g, chunk k -> cols [(g*NCHUNK+k)*NWG*8 : +NWG*8]
        ngrp = W // NWG
        gidx = np.zeros((P, ngrp * NCHUNK * NWG * 8), np.int16)
        for g in range(ngrp):
            for k in range(NCHUNK):
                flat = idx16[g * NWG:(g + 1) * NWG, k, :].reshape(-1)
                gidx[:, (g * NCHUNK + k) * NWG * 8:(g * NCHUNK + k + 1) * NWG * 8] = \
                    _wrap16(flat)
        # flush idx: group f -> cols [f*NWF*8 : +NWF*8]
        nfg = W // NWF
        fidx = np.zeros((P, nfg * NWF * 8), np.int16)
        for f in range(nfg):
            flat = destid[f * NWF:(f + 1) * NWF, :].reshape(-1)
            fidx[:, f * NWF * 8:(f + 1) * NWF * 8] = _wrap16(flat)

        deg_winT_dev = np.ascontiguousarray(deg_winT.T)    # [P, W]
        deg_winT0_dev = np.ascontiguousarray(deg_winT0.T)  # [P, W]
        tmp = np.ones(dsh_pad, np.float32)
        tmp[:dsh] = deg[c * dsh:(c + 1) * dsh].astype(np.float32)
        deg_own = np.ascontiguousarray(tmp.reshape(dsh_pad // P, P).T)
        out.append(dict(gidx=gidx, fidx=fidx, rel=rel_dev,
                        deg_winT=deg_winT_dev, deg_winT0=deg_winT0_dev,
                        deg_own=deg_own, n_win=len(wins)))
    return out, W, dsh, dsh_pad


# ------------------------------------------------------------- bass program

def build(n_nodes, n_cores, W, dsh, dsh_pad, in_ch, hid, out_ch,
          has_bias=False, debug=False):
    NT = dsh_pad // P          # node tiles per shard
    KT = in_ch // P            # k chunks for x @ W1
    ntab = n_cores * dsh_pad   # padded global table rows
    csz = ntab // NCHUNK
    hid_pad = 2 * hid          # tab1 rows padded to 256 B (bf16)
    ngrp = W // NWG
    nfg = W // NWF

    nc = bacc.Bacc("TRN2", target_bir_lowering=False, debug=False,
                   enable_asserts=False, num_devices=n_cores)

    f32, bf16 = mybir.dt.float32, mybir.dt.bfloat16
    i16 = mybir.dt.int16

    # kernel I/O
    xT = nc.dram_tensor("xT", [in_ch, dsh_pad], f32, kind="ExternalInput")
    W1_in = nc.dram_tensor("W1", [in_ch, hid], f32, kind="ExternalInput")
    W2_in = nc.dram_tensor("W2", [hid, out_ch], f32, kind="ExternalInput")
    b1_in = nc.dram_tensor("b1", [P, hid], f32, kind="ExternalInput")
    b2_in = nc.dram_tensor("b2", [P, out_ch], f32, kind="ExternalInput")
    fcwT_in = nc.dram_tensor("fcwT", [out_ch, 2], f32, kind="ExternalInput")
    fcb_in = nc.dram_tensor("fcb", [1, 2], f32, kind="ExternalInput")
    gidx_in = nc.dram_tensor("gidx", [P, ngrp * NCHUNK * NWG * 8], i16,
                             kind="ExternalInput")
    fidx_in = nc.dram_tensor("fidx", [P, nfg * NWF * 8], i16,
                             kind="ExternalInput")
    rel_in = nc.dram_tensor("rel", [P, W * TPW], bf16, kind="ExternalInput")
    deg_winT_in = nc.dram_tensor("deg_winT", [P, W], f32, kind="ExternalInput")
    deg_winT0_in = nc.dram_tensor("deg_winT0", [P, W], f32, kind="ExternalInput")
    deg_own_in = nc.dram_tensor("deg_own", [P, NT], f32, kind="ExternalInput")
    iota_in = nc.dram_tensor("iota", [P, SLOTS], bf16, kind="ExternalInput")
    ident_in = nc.dram_tensor("ident", [P, P], bf16, kind="ExternalInput")
    ones_in = nc.dram_tensor("ones", [P, 1], bf16, kind="ExternalInput")
    out_t = nc.dram_tensor("out", [1, 2], f32, kind="ExternalOutput")
    if debug:
        dbg_tab1 = nc.dram_tensor("dbg_tab1", [dsh_pad, 2 * hid], STAGE_DT,
                                  kind="ExternalOutput")
        dbg_h1 = nc.dram_tensor("dbg_h1", [dsh_pad + P, 2 * hid], STAGE_DT,
                                kind="ExternalOutput")
        dbg_tab2 = nc.dram_tensor("dbg_tab2", [dsh_pad, out_ch], STAGE_DT,
                                  kind="ExternalOutput")
        dbg_gsum = nc.dram_tensor("dbg_gsum", [P, 1], f32, kind="ExternalOutput")

    with tile.TileContext(nc) as tc:
        with tc.tile_pool(name="dram", bufs=1, space="DRAM") as dram, \
             tc.tile_pool(name="const", bufs=1) as cpool:

            # ---------------- persistent DRAM buffers
            tab1_shard = dram.tile([dsh_pad, hid_pad], STAGE_DT)
            tab2_shard = dram.tile([dsh_pad, out_ch], STAGE_DT)
            tab1_full = dram.tile([ntab, hid_pad], STAGE_DT)
            tab2_full = dram.tile([ntab, out_ch], STAGE_DT)
            h1_shard = dram.tile([dsh_pad + P, hid_pad], STAGE_DT)
            gsum_in_d = dram.tile([P, 1], f32)
            gsum_out_d = dram.tile([P, 1], f32)

            # ---------------- constants / per-node data in SBUF
            fidx_sb = cpool.tile([P, nfg * NWF * 8], i16)
            nc.sync.dma_start(fidx_sb[:], fidx_in.ap())
            rel_sb = cpool.tile([P, W * TPW], bf16)
            nc.sync.dma_start(rel_sb[:], rel_in.ap())
            iota_sb = cpool.tile([P, SLOTS], bf16)
            nc.sync.dma_start(iota_sb[:], iota_in.ap())
            ident_sb = cpool.tile([P, P], bf16)
            nc.sync.dma_start(ident_sb[:], ident_in.ap())
            ones_sb = cpool.tile([P, 1], bf16)
            nc.sync.dma_start(ones_sb[:], ones_in.ap())
            zeros_sb = cpool.tile([P, hid], bf16)
            nc.vector.memset(zeros_sb[:], 0.0)

            deg_winT_sb = cpool.tile([P, W], f32)
            nc.sync.dma_start(deg_winT_sb[:], deg_winT_in.ap())
            dinv_winT = cpool.tile([P, W], f32)
            nc.vector.reciprocal(dinv_winT[:], deg_winT_sb[:])
            nc.scalar.activation(out=dinv_winT[:], in_=dinv_winT[:],
                                 func=mybir.ActivationFunctionType.Sqrt)
            if has_bias:
                deg_winT0_sb = cpool.tile([P, W], f32)
                nc.sync.dma_start(deg_winT0_sb[:], deg_winT0_in.ap())
                u_winT = cpool.tile([P, W], f32)
                nc.scalar.activation(out=u_winT[:], in_=deg_winT0_sb[:],
                                     func=mybir.ActivationFunctionType.Sqrt)
            deg_own_sb = cpool.tile([P, NT], f32)
            nc.sync.dma_start(deg_own_sb[:], deg_own_in.ap())
            dinv_own = cpool.tile([P, NT], f32)
            nc.vector.reciprocal(dinv_own[:], deg_own_sb[:])
            nc.scalar.activation(out=dinv_own[:], in_=dinv_own[:],
                                 func=mybir.ActivationFunctionType.Sqrt)

            W1_sb = cpool.tile([P, KT, hid], bf16)
            nc.gpsimd.dma_start(W1_sb[:], W1_in.ap().rearrange(
                "(k p) h -> p k h", p=P))
            W2_sb = cpool.tile([hid, out_ch], bf16)
            nc.gpsimd.dma_start(W2_sb[:], W2_in.ap())
            if has_bias:
                b1_sb = cpool.tile([P, hid], bf16)
                nc.gpsimd.dma_start(b1_sb[:], b1_in.ap())
                b2_sb = cpool.tile([P, out_ch], bf16)
                nc.gpsimd.dma_start(b2_sb[:], b2_in.ap())
            else:
                b1_sb = b2_sb = None

            # ---------------- phase 0: zero h1_shard (scatter_add target)
            with tc.tile_pool(name="ph0", bufs=1) as ph0:
                zrow = ph0.tile([P, (dsh_pad + P) // P * hid_pad], bf16)
                nc.gpsimd.memset(zrow[:], 0.0)
                nc.sync.dma_start(
                    h1_shard[:].rearrange("(a p) f -> p a f", p=P),
                    zrow[:].rearrange("p (a f) -> p a f", f=hid_pad))

            # ---------------- phase 1: tab1 = (x @ W1) * dinv  (own shard)
            with tc.tile_pool(name="ph1", bufs=3) as ph1, \
                 tc.tile_pool(name="ph1ps", bufs=4, space="PSUM") as ph1ps:
                for j in range(NT):
                    xt = ph1.tile([P, KT, P], bf16, tag="xt")
                    nc.gpsimd.dma_start(
                        xt[:], xT.ap().rearrange("(k p) n -> p k n", p=P)
                        [:, :, j * P:(j + 1) * P])
                    ps = ph1ps.tile([P, hid], f32)
                    for k in range(KT):
                        nc.tensor.matmul(out=ps[:], lhsT=xt[:, k, :],
                                         rhs=W1_sb[:, k, :],
                                         start=(k == 0), stop=(k == KT - 1))
                    st = ph1.tile([P, hid], STAGE_DT, tag="st")
                    nc.scalar.activation(out=st[:], in_=ps[:],
                                         func=mybir.ActivationFunctionType.Copy,
                                         scale=dinv_own[:, j:j + 1])
                    nc.sync.dma_start(
                        tab1_shard[j * P:(j + 1) * P, 0:hid], st[:])
                    nc.sync.dma_start(
                        tab1_shard[j * P:(j + 1) * P, hid:hid_pad], zeros_sb[:])

            nc.gpsimd.collective_compute(
                "AllGather", mybir.AluOpType.bypass,
                replica_groups=[list(range(n_cores))],
                ins=[tab1_shard.opt()],
                outs=[tab1_full.opt()],
            )
            if debug:
                nc.sync.dma_start(dbg_tab1.ap(), tab1_shard[:])

            # ---------------- shared aggregation machinery
            def agg_layer(tab_full, rowlen, feat, bias_sb, out_cb):
                """Windows of TPW chunk-tiles; psum[d, f] per window handed to
                out_cb(w, psum_tile)."""
                with tc.tile_pool(name="agg", bufs=3) as ap_, \
                     tc.tile_pool(name="aggps", bufs=6, space="PSUM") as aps:
                    for g in range(ngrp):
                        w0 = g * NWG
                        gi = ap_.tile([P, NCHUNK, NWG * 8], i16, tag="gi",
                                      name=f"gi_{g}")
                        nc.sync.dma_start(
                            gi[:], gidx_in.ap()
                            [:, g * NCHUNK * NWG * 8:(g + 1) * NCHUNK * NWG * 8]
                            .rearrange("p (k x) -> p k x", k=NCHUNK))
                        gts = []
                        for k in range(NCHUNK):
                            gt = ap_.tile([P, NWG, rowlen], bf16, tag=f"gt{k}",
                                          name=f"gt{k}_{g}")
                            nc.gpsimd.dma_gather(
                                gt[:],
                                tab_full[k * csz:(k + 1) * csz, :],
                                gi[:, k, :],
                                NWG * P, NWG * P, rowlen,
                                single_packet=False)
                            gts.append(gt)
                        for wi in range(NWG):
                            w = w0 + wi
                            m = ap_.tile([P, SLOTS], bf16, tag="m",
                                         name=f"m_{w}")
                            nc.vector.tensor_tensor(
                                out=m[:].rearrange("p (t d) -> p t d", d=P),
                                in0=iota_sb[:].rearrange("p (t d) -> p t d", d=P),
                                in1=rel_sb[:, w * TPW:(w + 1) * TPW, None]
                                    .to_broadcast([P, TPW, P]),
                                op=mybir.AluOpType.is_equal)
                            ps = aps.tile([P, feat], f32, tag="ps",
                                          name=f"ps_{w}")
                            for k in range(NCHUNK):
                                nc.tensor.matmul(
                                    out=ps[:], lhsT=m[:, k * P:(k + 1) * P],
                                    rhs=gts[k][:, wi, 0:feat],
                                    start=(k == 0),
                                    stop=(k == NCHUNK - 1 and not has_bias))
                            if has_bias:
                                xd = ap_.tile([P, P], bf16, tag="xd",
                                              name=f"xd_{w}")
                                nc.vector.tensor_scalar_mul(
                                    xd[:], ident_sb[:], u_winT[:, w:w + 1])
                                nc.tensor.matmul(
                                    out=ps[:], lhsT=xd[:], rhs=bias_sb[:, :],
                                    start=False, stop=True)
                            out_cb(w, ps)

            # ---------------- phase 2: layer-1 aggregation
            l1_stage = {}
            with tc.tile_pool(name="l1st", bufs=2) as l1p:
                def l1_cb(w, ps):
                    wi = w % NWF
                    if wi == 0:
                        l1_stage["t"] = l1p.tile([P, NWF, hid], STAGE_DT,
                                                 tag="l1stage",
                                                 name=f"l1stage_{w}")
                    nc.scalar.activation(out=l1_stage["t"][:, wi, :], in_=ps[:],
                                         func=mybir.ActivationFunctionType.Relu,
                                         scale=dinv_winT[:, w:w + 1])
                    if wi == NWF - 1:
                        f = w // NWF
                        nc.gpsimd.dma_scatter_add(
                            h1_shard[:, 0:hid],
                            l1_stage["t"][:],
                            fidx_sb[:, f * NWF * 8:(f + 1) * NWF * 8],
                            NWF * P, NWF * P, hid, elem_step=hid_pad,
                            single_packet=False)

                agg_layer(tab1_full, hid_pad, hid, b1_sb, l1_cb)

            # ---------------- phase 3: tab2 = (h1 @ W2) * dinv
            with tc.tile_pool(name="ph3", bufs=3) as ph3, \
                 tc.tile_pool(name="ph3ps", bufs=3, space="PSUM") as ph3ps:
                for j in range(NT):
                    ht = ph3.tile([P, hid], STAGE_DT, tag="ht")
                    nc.sync.dma_start(ht[:], h1_shard[j * P:(j + 1) * P, 0:hid])
                    pst = ph3ps.tile([hid, P], STAGE_DT, tag="pst")
                    nc.tensor.transpose(out=pst[:], in_=ht[:],
                                        identity=ident_sb[:])
                    htT = ph3.tile([hid, P], STAGE_DT, tag="htT")
                    nc.vector.tensor_copy(htT[:], pst[:])
                    ps2 = ph3ps.tile([P, out_ch], f32, tag="ps2")
                    nc.tensor.matmul(out=ps2[:], lhsT=htT[:], rhs=W2_sb[:],
                                     start=True, stop=True)
                    st = ph3.tile([P, out_ch], STAGE_DT, tag="st2")
                    nc.scalar.activation(out=st[:], in_=ps2[:],
                                         func=mybir.ActivationFunctionType.Copy,
                                         scale=dinv_own[:, j:j + 1])
                    nc.sync.dma_start(tab2_shard[j * P:(j + 1) * P, :], st[:])

            if debug:
                nc.sync.dma_start(dbg_h1.ap(), h1_shard[:])
                nc.sync.dma_start(dbg_tab2.ap(), tab2_shard[:])
            nc.gpsimd.collective_compute(
                "AllGather", mybir.AluOpType.bypass,
                replica_groups=[list(range(n_cores))],
                ins=[tab2_shard.opt()],
                outs=[tab2_full.opt()],
            )

            # ---------------- phase 4: layer-2 aggregation + mean accumulation
            with tc.tile_pool(name="gsump", bufs=1, space="PSUM") as gsp, \
                 tc.tile_pool(name="l2st", bufs=3) as l2p:
                gsum_ps = gsp.tile([out_ch, 1], f32)

                def l2_cb(w, ps):
                    st = l2p.tile([P, out_ch], STAGE_DT, tag="h2",
                                  name=f"h2_{w}")
                    nc.scalar.activation(out=st[:], in_=ps[:],
                                         func=mybir.ActivationFunctionType.Relu,
                                         scale=dinv_winT[:, w:w + 1])
                    nc.tensor.matmul(out=gsum_ps[:], lhsT=st[:], rhs=ones_sb[:],
                                     start=(w == 0), stop=(w == W - 1))

                agg_layer(tab2_full, out_ch, out_ch, b2_sb, l2_cb)

                gsum_sb = l2p.tile([out_ch, 1], f32, tag="gs")
                nc.vector.tensor_copy(gsum_sb[:], gsum_ps[:])
                nc.sync.dma_start(gsum_in_d[:], gsum_sb[:])

            if debug:
                nc.sync.dma_start(dbg_gsum.ap(), gsum_in_d[:])
            nc.gpsimd.collective_compute(
                "AllReduce", mybir.AluOpType.add,
                replica_groups=[list(range(n_cores))],
                ins=[gsum_in_d.opt()], outs=[gsum_out_d.opt()],
            )

            # ---------------- phase 5: final FC
            with tc.tile_pool(name="fin", bufs=1) as fin, \
                 tc.tile_pool(name="finps", bufs=1, space="PSUM") as finps:
                gsum2 = fin.tile([out_ch, 1], f32)
                nc.sync.dma_start(gsum2[:], gsum_out_d[:])
                gmean = fin.tile([out_ch, 1], f32)
                nc.vector.tensor_scalar_mul(gmean[:], gsum2[:], 1.0 / n_nodes)
                fcwT_sb = fin.tile([out_ch, 2], f32)
                nc.sync.dma_start(fcwT_sb[:], fcwT_in.ap())
                ops = finps.tile([1, 2], f32)
                nc.tensor.matmul(out=ops[:], lhsT=gmean[:], rhs=fcwT_sb[:],
                                 start=True, stop=True)
                fcb_sb = fin.tile([1, 2], f32)
                nc.sync.dma_start(fcb_sb[:], fcb_in.ap())
                res = fin.tile([1, 2], f32)
                nc.vector.tensor_tensor(out=res[:], in0=ops[:], in1=fcb_sb[:],
                                        op=mybir.AluOpType.add)
                nc.sync.dma_start(out_t.ap(), res[:])

    nc.compile()
    return nc


# ----------------------------------------------------------------- interface

_CACHE = {}


def _run(inputs, n_cores=N_CORES, trace=False, trace_kwargs=None, debug=False):
    x = np.asarray(inputs["x"], np.float32)
    edge_index = np.asarray(inputs["edge_index"])
    n_nodes, in_ch = x.shape
    hid = np.asarray(inputs["W1"]).shape[1]
    out_ch = np.asarray(inputs["W2"]).shape[1]

    b1v = np.asarray(inputs["b1"], np.float32).reshape(-1)
    b2v = np.asarray(inputs["b2"], np.float32).reshape(-1)
    has_bias = bool(b1v.any() or b2v.any())
    key = (n_nodes, edge_index.shape[1], in_ch, hid, out_ch, n_cores,
           has_bias, debug)
    if key not in _CACHE:
        cores, W, dsh, dsh_pad = prep(edge_index, n_nodes, n_cores)
        nc = build(n_nodes, n_cores, W, dsh, dsh_pad, in_ch, hid, out_ch,
                   has_bias=has_bias, debug=debug)
        _CACHE[key] = (nc, cores, W, dsh, dsh_pad)
    nc, cores, W, dsh, dsh_pad = _CACHE[key]

    iota = np.tile(np.arange(P, dtype=np.float32), TPW)[None, :].repeat(P, 0)
    common = {
        "W1": np.asarray(inputs["W1"], np.float32),
        "W2": np.asarray(inputs["W2"], np.float32),
        "b1": np.broadcast_to(b1v[None, :], (P, b1v.size)).copy(),
        "b2": np.broadcast_to(b2v[None, :], (P, b2v.size)).copy(),
        "fcwT": np.ascontiguousarray(np.asarray(inputs["fcW"], np.float32).T),
        "fcb": np.asarray(inputs["fcb"], np.float32).reshape(1, -1),
        "iota": iota.astype(STAGE_NP),
        "ident": np.eye(P, dtype=STAGE_NP),
        "ones": np.ones((P, 1), STAGE_NP),
    }
    in_maps = []
    for c in range(n_cores):
        cd = cores[c]
        xs = np.zeros((in_ch, dsh_pad), np.float32)
        xs[:, :dsh] = x[c * dsh:(c + 1) * dsh, :].T
        in_maps.append({
            **common,
            "xT": xs,
            "gidx": cd["gidx"],
            "fidx": cd["fidx"],
            "rel": cd["rel"],
            "deg_winT": cd["deg_winT"],
            "deg_winT0": cd["deg_winT0"],
            "deg_own": cd["deg_own"],
        })
    res = bass_utils.run_bass_kernel_spmd(
        nc, in_maps, core_ids=list(range(n_cores)),
        trace=trace, trace_kwargs=trace_kwargs or {})
    out = res.results[0]["out"].reshape(2).astype(np.float32)
    return out, res


def kernel(**inputs) -> np.ndarray:
    out, _ = _run(inputs)
    return out


# revision 12
# speedup vs baseline: 1.1023x; 1.1023x over previous
"""Trainium2 Bass kernel for a 2-layer GCN (GCNConv -> ReLU -> GCNConv -> ReLU ->
mean-pool -> FC) on a 100k-node / 3.2M-edge graph, SPMD across 8 NeuronCores.

Sharding: destination sharding. Edges (plus self-loops) are sorted by
destination; core c owns destinations [c*12500, (c+1)*12500). Consecutive
destinations are greedily packed into "windows" of at most 128 destinations
and 4 tiles of 128 edge slots; tile k of a window holds the window's edges
whose source row lives in table chunk k (the global padded table is split in
4 chunks of <= 32768 rows so dma_gather's int16 indices reach every row).
Per window the kernel:
  - gathers 4x128 source-node table rows (256-byte rows, bf16) with batched
    `dma_gather` calls (one per chunk per NWG windows),
  - builds a one-hot selection matrix M[e, d] = (rel[e] == d) on the DVE
    (iota vs per-slot rel compare),
  - accumulates psum[d, f] = sum_e M[e,d] * G[e,f] with 4 PE matmuls into
    one PSUM bank, optionally adds the bias as a diag(u) matmul,
  - applies relu(dinv[d] * psum) with one ACT instruction (per-partition
    scale), and
  - scatters the finished rows back to HBM with `dma_scatter_add` into a
    pre-zeroed buffer (layer 1) or accumulates the global feature sum with a
    ones-matmul (layer 2).
Node tables are stored pre-scaled by dinv[src], so edge contributions are
plain sums; self-loops are ordinary edges. Tables are exchanged between
layers with an AllGather; the final [2]-vector FC runs on-device after an
AllReduce of the feature sums.
"""

import numpy as np
import ml_dtypes

import concourse.bass as bass
import concourse.bacc as bacc
import concourse.mybir as mybir
import concourse.tile as tile
import concourse.bass_utils as bass_utils

P = 128
N_CORES = 8
NCHUNK = 4              # table chunks (int16 index reach)

STAGE_DT = mybir.dt.bfloat16
STAGE_NP = ml_dtypes.bfloat16

TPW = NCHUNK            # tiles per window (one per chunk)
SLOTS = TPW * P         # edge slots per window
NWG = 16                # windows per gather batch
NWF = 16                # windows per flush-scatter batch


def _wrap16(flat):
    """dma_gather/scatter_add index layout: idx i -> partition i%16,
    col i//16, replicated across the eight 16-partition groups."""
    n = flat.size
    assert n % 16 == 0
    arr = np.ascontiguousarray(flat.reshape(n // 16, 16).T)
    return np.tile(arr, (8, 1)).astype(np.int16)


# ----------------------------------------------------------------- host prep

def prep(edge_index, n_nodes, n_cores):
    """Build per-core index tensors. Pure index manipulation (no float math
    on the graph signal)."""
    dsh = n_nodes // n_cores
    assert dsh * n_cores == n_nodes
    dsh_pad = ((dsh + P - 1) // P) * P
    ntab = n_cores * dsh_pad
    assert ntab % NCHUNK == 0
    csz = ntab // NCHUNK
    assert csz <= 32767

    src = edge_index[0].astype(np.int64)
    dst = edge_index[1].astype(np.int64)
    deg = np.bincount(dst, minlength=n_nodes).astype(np.int64) + 1  # + self loop

    loops = np.arange(n_nodes, dtype=np.int64)
    src_a = np.concatenate([src, loops])
    dst_a = np.concatenate([dst, loops])
    order = np.argsort(dst_a, kind="stable")
    src_g = src_a[order]
    dst_g = dst_a[order]
    # global row id in the AllGather-concatenated (padded) table layout
    gid_g = (src_g // dsh) * dsh_pad + (src_g % dsh)

    pre = []
    for c in range(n_cores):
        lo, hi = c * dsh, (c + 1) * dsh
        a = np.searchsorted(dst_g, lo)
        b = np.searchsorted(dst_g, hi)
        gid = gid_g[a:b]
        d = dst_g[a:b] - lo
        ck = gid // csz
        # per (dest, chunk) counts
        cnt = np.bincount(d * NCHUNK + ck, minlength=dsh * NCHUNK).reshape(dsh, NCHUNK)
        # greedy window packing: <= P dests, <= P edges per chunk
        wins = []
        d0 = 0
        run = np.zeros(NCHUNK, np.int64)
        while d0 < dsh:
            run[:] = 0
            dd = d0
            while dd < dsh and (dd - d0) < P and np.all(run + cnt[dd] <= P):
                run += cnt[dd]
                dd += 1
            if dd == d0:
                raise ValueError(f"dest {d0} chunk degree {cnt[d0].max()} > {P}")
            wins.append((d0, dd))
            d0 = dd
        pre.append((wins, gid, d, ck))

    W = max(len(p[0]) for p in pre)
    W = ((W + max(NWG, NWF) - 1) // max(NWG, NWF)) * max(NWG, NWF)

    out = []
    for c in range(n_cores):
        wins, gid, d, ck = pre[c]
        bounds = np.searchsorted(d, np.arange(dsh + 1))
        idx16 = np.zeros((W, NCHUNK, P), np.int16)       # pad -> row 0 of chunk
        rel = np.full((W, NCHUNK, P), -1.0, np.float32)  # pad -> -1
        destid = np.zeros((W, P), np.int64)
        deg_winT = np.ones((W, P), np.float32)
        deg_winT0 = np.zeros((W, P), np.float32)
        for w, (d0, d1) in enumerate(wins):
            e0, e1 = bounds[d0], bounds[d1]
            gw, dw, cw = gid[e0:e1], d[e0:e1], ck[e0:e1]
            for k in range(NCHUNK):
                m = cw == k
                n = int(m.sum())
                idx16[w, k, :n] = (gw[m] - k * csz).astype(np.int16)
                rel[w, k, :n] = (dw[m] - d0).astype(np.float32)
            nd = d1 - d0
            destid[w, :nd] = d0 + np.arange(nd)
            destid[w, nd:] = dsh_pad + (np.arange(P - nd) % P)
            dg = deg[c * dsh + d0: c * dsh + d1].astype(np.float32)
            deg_winT[w, :nd] = dg
            deg_winT0[w, :nd] = dg
        for w in range(len(wins), W):
            destid[w, :] = dsh_pad + (np.arange(P) % P)

        # device layouts -----------------------------------------------------
        # rel: slot p of tile k of window w -> [p, w*TPW + k]
        rel_dev = np.ascontiguousarray(
            rel.transpose(2, 0, 1).reshape(P, W * TPW)).astype(STAGE_NP)
        # gather idx: group g, chunk k -> cols [(g*NCHUNK+k)*NWG*8 : +NWG*8]
        ngrp = W // NWG
        gidx = np.zeros((P, ngrp * NCHUNK * NWG * 8), np.int16)
        for g in range(ngrp):
            for k in range(NCHUNK):
                flat = idx16[g * NWG:(g + 1) * NWG, k, :].reshape(-1)
                gidx[:, (g * NCHUNK + k) * NWG * 8:(g * NCHUNK + k + 1) * NWG * 8] = \
                    _wrap16(flat)
        # flush idx: group f -> cols [f*NWF*8 : +NWF*8]
        nfg = W // NWF
        fidx = np.zeros((P, nfg * NWF * 8), np.int16)
        for f in range(nfg):
            flat = destid[f * NWF:(f + 1) * NWF, :].reshape(-1)
            fidx[:, f * NWF * 8:(f + 1) * NWF * 8] = _wrap16(flat)

        deg_winT_dev = np.ascontiguousarray(deg_winT.T)    # [P, W]
        deg_winT0_dev = np.ascontiguousarray(deg_winT0.T)  # [P, W]
        tmp = np.ones(dsh_pad, np.float32)
        tmp[:dsh] = deg[c * dsh:(c + 1) * dsh].astype(np.float32)
        deg_own = np.ascontiguousarray(tmp.reshape(dsh_pad // P, P).T)
        out.append(dict(gidx=gidx, fidx=fidx, rel=rel_dev,
                        deg_winT=deg_winT_dev, deg_winT0=deg_winT0_dev,
                        deg_own=deg_own, n_win=len(wins)))
    return out, W, dsh, dsh_pad


# ------------------------------------------------------------- bass program

def build(n_nodes, n_cores, W, dsh, dsh_pad, in_ch, hid, out_ch,
          has_bias=False, debug=False, sim_single=False):
    NT = dsh_pad // P          # node tiles per shard
    KT = in_ch // P            # k chunks for x @ W1
    ntab = n_cores * dsh_pad   # padded global table rows
    csz = ntab // NCHUNK
    hid_pad = 2 * hid          # tab1 rows padded to 256 B (bf16)
    ngrp = W // NWG
    nfg = W // NWF

    nc = bacc.Bacc("TRN2", target_bir_lowering=False, debug=False,
                   enable_asserts=False,
                   num_devices=1 if sim_single else n_cores)

    f32, bf16 = mybir.dt.float32, mybir.dt.bfloat16
    i16 = mybir.dt.int16

    # kernel I/O
    xT = nc.dram_tensor("xT", [in_ch, dsh_pad], f32, kind="ExternalInput")
    W1_in = nc.dram_tensor("W1", [in_ch, hid], f32, kind="ExternalInput")
    W2_in = nc.dram_tensor("W2", [hid, out_ch], f32, kind="ExternalInput")
    b1_in = nc.dram_tensor("b1", [P, hid], f32, kind="ExternalInput")
    b2_in = nc.dram_tensor("b2", [P, out_ch], f32, kind="ExternalInput")
    fcwT_in = nc.dram_tensor("fcwT", [out_ch, 2], f32, kind="ExternalInput")
    fcb_in = nc.dram_tensor("fcb", [1, 2], f32, kind="ExternalInput")
    gidx_in = nc.dram_tensor("gidx", [P, ngrp * NCHUNK * NWG * 8], i16,
                             kind="ExternalInput")
    fidx_in = nc.dram_tensor("fidx", [P, nfg * NWF * 8], i16,
                             kind="ExternalInput")
    rel_in = nc.dram_tensor("rel", [P, W * TPW], bf16, kind="ExternalInput")
    deg_winT_in = nc.dram_tensor("deg_winT", [P, W], f32, kind="ExternalInput")
    deg_winT0_in = nc.dram_tensor("deg_winT0", [P, W], f32, kind="ExternalInput")
    deg_own_in = nc.dram_tensor("deg_own", [P, NT], f32, kind="ExternalInput")
    iota_in = nc.dram_tensor("iota", [P, SLOTS], bf16, kind="ExternalInput")
    ident_in = nc.dram_tensor("ident", [P, P], bf16, kind="ExternalInput")
    ones_in = nc.dram_tensor("ones", [P, 1], bf16, kind="ExternalInput")
    out_t = nc.dram_tensor("out", [1, 2], f32, kind="ExternalOutput")
    if debug:
        dbg_tab1 = nc.dram_tensor("dbg_tab1", [dsh_pad, 2 * hid], STAGE_DT,
                                  kind="ExternalOutput")
        dbg_h1 = nc.dram_tensor("dbg_h1", [dsh_pad + P, 2 * hid], STAGE_DT,
                                kind="ExternalOutput")
        dbg_tab2 = nc.dram_tensor("dbg_tab2", [dsh_pad, out_ch], STAGE_DT,
                                  kind="ExternalOutput")
        dbg_gsum = nc.dram_tensor("dbg_gsum", [P, 1], f32, kind="ExternalOutput")

    with tile.TileContext(nc) as tc:
        with tc.tile_pool(name="dram", bufs=1, space="DRAM") as dram, \
             tc.tile_pool(name="const", bufs=1) as cpool:

            # ---------------- persistent DRAM buffers
            tab1_shard = dram.tile([dsh_pad, hid_pad], STAGE_DT)
            tab2_shard = dram.tile([dsh_pad, out_ch], STAGE_DT)
            tab1_full = dram.tile([ntab, hid_pad], STAGE_DT)
            tab2_full = dram.tile([ntab, out_ch], STAGE_DT)
            h1_shard = dram.tile([dsh_pad + P, hid_pad], STAGE_DT)
            gsum_in_d = dram.tile([P, 1], f32)
            gsum_out_d = dram.tile([P, 1], f32)

            # ---------------- constants / per-node data in SBUF
            fidx_sb = cpool.tile([P, nfg * NWF * 8], i16)
            nc.sync.dma_start(fidx_sb[:], fidx_in.ap())
            rel_sb = cpool.tile([P, W * TPW], bf16)
            nc.sync.dma_start(rel_sb[:], rel_in.ap())
            iota_sb = cpool.tile([P, SLOTS], bf16)
            nc.sync.dma_start(iota_sb[:], iota_in.ap())
            ident_sb = cpool.tile([P, P], bf16)
            nc.sync.dma_start(ident_sb[:], ident_in.ap())
            ones_sb = cpool.tile([P, 1], bf16)
            nc.sync.dma_start(ones_sb[:], ones_in.ap())
            zeros_sb = cpool.tile([P, hid], bf16)
            nc.vector.memset(zeros_sb[:], 0.0)

            deg_winT_sb = cpool.tile([P, W], f32)
            nc.sync.dma_start(deg_winT_sb[:], deg_winT_in.ap())
            dinv_winT = cpool.tile([P, W], f32)
            nc.vector.reciprocal(dinv_winT[:], deg_winT_sb[:])
            nc.scalar.activation(out=dinv_winT[:], in_=dinv_winT[:],
                                 func=mybir.ActivationFunctionType.Sqrt)
            if has_bias:
                deg_winT0_sb = cpool.tile([P, W], f32)
                nc.sync.dma_start(deg_winT0_sb[:], deg_winT0_in.ap())
                u_winT = cpool.tile([P, W], f32)
                nc.scalar.activation(out=u_winT[:], in_=deg_winT0_sb[:],
                                     func=mybir.ActivationFunctionType.Sqrt)
            deg_own_sb = cpool.tile([P, NT], f32)
            nc.sync.dma_start(deg_own_sb[:], deg_own_in.ap())
            dinv_own = cpool.tile([P, NT], f32)
            nc.vector.reciprocal(dinv_own[:], deg_own_sb[:])
            nc.scalar.activation(out=dinv_own[:], in_=dinv_own[:],
                                 func=mybir.ActivationFunctionType.Sqrt)

            W1_sb = cpool.tile([P, KT, hid], bf16)
            nc.gpsimd.dma_start(W1_sb[:], W1_in.ap().rearrange(
                "(k p) h -> p k h", p=P))
            W2_sb = cpool.tile([hid, out_ch], bf16)
            nc.gpsimd.dma_start(W2_sb[:], W2_in.ap())
            if has_bias:
                b1_sb = cpool.tile([P, hid], bf16)
                nc.gpsimd.dma_start(b1_sb[:], b1_in.ap())
                b2_sb = cpool.tile([P, out_ch], bf16)
                nc.gpsimd.dma_start(b2_sb[:], b2_in.ap())
            else:
                b1_sb = b2_sb = None

            # ---------------- phase 0: zero h1_shard (scatter_add target)
            with tc.tile_pool(name="ph0", bufs=1) as ph0:
                zrow = ph0.tile([P, (dsh_pad + P) // P * hid_pad], bf16)
                nc.gpsimd.memset(zrow[:], 0.0)
                nc.sync.dma_start(
                    h1_shard[:].rearrange("(a p) f -> p a f", p=P),
                    zrow[:].rearrange("p (a f) -> p a f", f=hid_pad))

            # ---------------- phase 1: tab1 = (x @ W1) * dinv  (own shard)
            with tc.tile_pool(name="ph1", bufs=3) as ph1, \
                 tc.tile_pool(name="ph1ps", bufs=4, space="PSUM") as ph1ps:
                for j in range(NT):
                    xt = ph1.tile([P, KT, P], bf16, tag="xt")
                    nc.gpsimd.dma_start(
                        xt[:], xT.ap().rearrange("(k p) n -> p k n", p=P)
                        [:, :, j * P:(j + 1) * P])
                    ps = ph1ps.tile([P, hid], f32)
                    for k in range(KT):
                        nc.tensor.matmul(out=ps[:], lhsT=xt[:, k, :],
                                         rhs=W1_sb[:, k, :],
                                         start=(k == 0), stop=(k == KT - 1))
                    st = ph1.tile([P, hid], STAGE_DT, tag="st")
                    nc.scalar.activation(out=st[:], in_=ps[:],
                                         func=mybir.ActivationFunctionType.Copy,
                                         scale=dinv_own[:, j:j + 1])
                    nc.sync.dma_start(
                        tab1_shard[j * P:(j + 1) * P, 0:hid], st[:])
                    nc.sync.dma_start(
                        tab1_shard[j * P:(j + 1) * P, hid:hid_pad], zeros_sb[:])

            if sim_single:
                nc.sync.dma_start(tab1_full[0:dsh_pad, :], tab1_shard[:])
            else:
                nc.gpsimd.collective_compute(
                    "AllGather", mybir.AluOpType.bypass,
                    replica_groups=[list(range(n_cores))],
                    ins=[tab1_shard.opt()],
                    outs=[tab1_full.opt()],
                )
            if debug:
                nc.sync.dma_start(dbg_tab1.ap(), tab1_shard[:])

            # ---------------- shared aggregation machinery
            def agg_layer(tab_full, rowlen, feat, bias_sb, out_cb):
                """Windows of TPW chunk-tiles; psum[d, f] per window handed to
                out_cb(w, psum_tile)."""
                with tc.tile_pool(name="agg", bufs=3) as ap_, \
                     tc.tile_pool(name="aggps", bufs=6, space="PSUM") as aps:
                    for g in range(ngrp):
                        w0 = g * NWG
                        gi = ap_.tile([P, NCHUNK, NWG * 8], i16, tag="gi",
                                      name=f"gi_{g}")
                        nc.sync.dma_start(
                            gi[:], gidx_in.ap()
                            [:, g * NCHUNK * NWG * 8:(g + 1) * NCHUNK * NWG * 8]
                            .rearrange("p (k x) -> p k x", k=NCHUNK))
                        gts = []
                        for k in range(NCHUNK):
                            gt = ap_.tile([P, NWG, rowlen], bf16, tag=f"gt{k}",
                                          name=f"gt{k}_{g}")
                            nc.gpsimd.dma_gather(
                                gt[:],
                                tab_full[k * csz:(k + 1) * csz, :],
                                gi[:, k, :],
                                NWG * P, NWG * P, rowlen,
                                single_packet=False)
                            gts.append(gt)
                        for wi in range(NWG):
                            w = w0 + wi
                            m = ap_.tile([P, SLOTS], bf16, tag="m",
                                         name=f"m_{w}")
                            nc.vector.tensor_tensor(
                                out=m[:].rearrange("p (t d) -> p t d", d=P),
                                in0=iota_sb[:].rearrange("p (t d) -> p t d", d=P),
                                in1=rel_sb[:, w * TPW:(w + 1) * TPW, None]
                                    .to_broadcast([P, TPW, P]),
                                op=mybir.AluOpType.is_equal)
                            ps = aps.tile([P, feat], f32, tag="ps",
                                          name=f"ps_{w}")
                            for k in range(NCHUNK):
                                nc.tensor.matmul(
                                    out=ps[:], lhsT=m[:, k * P:(k + 1) * P],
                                    rhs=gts[k][:, wi, 0:feat],
                                    start=(k == 0),
                                    stop=(k == NCHUNK - 1 and not has_bias))
                            if has_bias:
                                xd = ap_.tile([P, P], bf16, tag="xd",
                                              name=f"xd_{w}")
                                nc.vector.tensor_scalar_mul(
                                    xd[:], ident_sb[:], u_winT[:, w:w + 1])
                                nc.tensor.matmul(
                                    out=ps[:], lhsT=xd[:], rhs=bias_sb[:, :],
                                    start=False, stop=True)
                            out_cb(w, ps)

            # ---------------- phase 2: layer-1 aggregation
            l1_stage = {}
            with tc.tile_pool(name="l1st", bufs=2) as l1p:
                def l1_cb(w, ps):
                    wi = w % NWF
                    if wi == 0:
                        l1_stage["t"] = l1p.tile([P, NWF, hid], STAGE_DT,
                                                 tag="l1stage",
                                                 name=f"l1stage_{w}")
                    nc.scalar.activation(out=l1_stage["t"][:, wi, :], in_=ps[:],
                                         func=mybir.ActivationFunctionType.Relu,
                                         scale=dinv_winT[:, w:w + 1])
                    if wi == NWF - 1:
                        f = w // NWF
                        nc.gpsimd.dma_scatter_add(
                            h1_shard[:, 0:hid],
                            l1_stage["t"][:],
                            fidx_sb[:, f * NWF * 8:(f + 1) * NWF * 8],
                            NWF * P, NWF * P, hid, elem_step=hid_pad,
                            single_packet=False)

                agg_layer(tab1_full, hid_pad, hid, b1_sb, l1_cb)

            # ---------------- phase 3: tab2 = (h1 @ W2) * dinv
            with tc.tile_pool(name="ph3", bufs=3) as ph3, \
                 tc.tile_pool(name="ph3ps", bufs=3, space="PSUM") as ph3ps:
                for j in range(NT):
                    ht = ph3.tile([P, hid], STAGE_DT, tag="ht")
                    nc.sync.dma_start(ht[:], h1_shard[j * P:(j + 1) * P, 0:hid])
                    pst = ph3ps.tile([hid, P], STAGE_DT, tag="pst")
                    nc.tensor.transpose(out=pst[:], in_=ht[:],
                                        identity=ident_sb[:])
                    htT = ph3.tile([hid, P], STAGE_DT, tag="htT")
                    nc.vector.tensor_copy(htT[:], pst[:])
                    ps2 = ph3ps.tile([P, out_ch], f32, tag="ps2")
                    nc.tensor.matmul(out=ps2[:], lhsT=htT[:], rhs=W2_sb[:],
                                     start=True, stop=True)
                    st = ph3.tile([P, out_ch], STAGE_DT, tag="st2")
                    nc.scalar.activation(out=st[:], in_=ps2[:],
                                         func=mybir.ActivationFunctionType.Copy,
                                         scale=dinv_own[:, j:j + 1])
                    nc.sync.dma_start(tab2_shard[j * P:(j + 1) * P, :], st[:])

            if debug:
                nc.sync.dma_start(dbg_h1.ap(), h1_shard[:])
                nc.sync.dma_start(dbg_tab2.ap(), tab2_shard[:])
            if sim_single:
                nc.sync.dma_start(tab2_full[0:dsh_pad, :], tab2_shard[:])
            else:
                nc.gpsimd.collective_compute(
                    "AllGather", mybir.AluOpType.bypass,
                    replica_groups=[list(range(n_cores))],
                    ins=[tab2_shard.opt()],
                    outs=[tab2_full.opt()],
                )

            # ---------------- phase 4: layer-2 aggregation + mean accumulation
            with tc.tile_pool(name="gsump", bufs=1, space="PSUM") as gsp, \
                 tc.tile_pool(name="l2st", bufs=3) as l2p:
                gsum_ps = gsp.tile([out_ch, 1], f32)

                def l2_cb(w, ps):
                    st = l2p.tile([P, out_ch], STAGE_DT, tag="h2",
                                  name=f"h2_{w}")
                    nc.scalar.activation(out=st[:], in_=ps[:],
                                         func=mybir.ActivationFunctionType.Relu,
                                         scale=dinv_winT[:, w:w + 1])
                    nc.tensor.matmul(out=gsum_ps[:], lhsT=st[:], rhs=ones_sb[:],
                                     start=(w == 0), stop=(w == W - 1))

                agg_layer(tab2_full, out_ch, out_ch, b2_sb, l2_cb)

                gsum_sb = l2p.tile([out_ch, 1], f32, tag="gs")
                nc.vector.tensor_copy(gsum_sb[:], gsum_ps[:])
                nc.sync.dma_start(gsum_in_d[:], gsum_sb[:])

            if debug:
                nc.sync.dma_start(dbg_gsum.ap(), gsum_in_d[:])
            if sim_single:
                nc.sync.dma_start(gsum_out_d[:], gsum_in_d[:])
            else:
                nc.gpsimd.collective_compute(
                    "AllReduce", mybir.AluOpType.add,
                    replica_groups=[list(range(n_cores))],
                    ins=[gsum_in_d.opt()], outs=[gsum_out_d.opt()],
                )

            # ---------------- phase 5: final FC
            with tc.tile_pool(name="fin", bufs=1) as fin, \
                 tc.tile_pool(name="finps", bufs=1, space="PSUM") as finps:
                gsum2 = fin.tile([out_ch, 1], f32)
                nc.sync.dma_start(gsum2[:], gsum_out_d[:])
                gmean = fin.tile([out_ch, 1], f32)
                nc.vector.tensor_scalar_mul(gmean[:], gsum2[:], 1.0 / n_nodes)
                fcwT_sb = fin.tile([out_ch, 2], f32)
                nc.sync.dma_start(fcwT_sb[:], fcwT_in.ap())
                ops = finps.tile([1, 2], f32)
                nc.tensor.matmul(out=ops[:], lhsT=gmean[:], rhs=fcwT_sb[:],
                                 start=True, stop=True)
                fcb_sb = fin.tile([1, 2], f32)
                nc.sync.dma_start(fcb_sb[:], fcb_in.ap())
                res = fin.tile([1, 2], f32)
                nc.vector.tensor_tensor(out=res[:], in0=ops[:], in1=fcb_sb[:],
                                        op=mybir.AluOpType.add)
                nc.sync.dma_start(out_t.ap(), res[:])

    nc.compile()
    return nc


# ----------------------------------------------------------------- interface

_CACHE = {}


def _run(inputs, n_cores=N_CORES, trace=False, trace_kwargs=None, debug=False):
    x = np.asarray(inputs["x"], np.float32)
    edge_index = np.asarray(inputs["edge_index"])
    n_nodes, in_ch = x.shape
    hid = np.asarray(inputs["W1"]).shape[1]
    out_ch = np.asarray(inputs["W2"]).shape[1]

    b1v = np.asarray(inputs["b1"], np.float32).reshape(-1)
    b2v = np.asarray(inputs["b2"], np.float32).reshape(-1)
    has_bias = bool(b1v.any() or b2v.any())
    key = (n_nodes, edge_index.shape[1], in_ch, hid, out_ch, n_cores,
           has_bias, debug)
    if key not in _CACHE:
        cores, W, dsh, dsh_pad = prep(edge_index, n_nodes, n_cores)
        nc = build(n_nodes, n_cores, W, dsh, dsh_pad, in_ch, hid, out_ch,
                   has_bias=has_bias, debug=debug)
        _CACHE[key] = (nc, cores, W, dsh, dsh_pad)
    nc, cores, W, dsh, dsh_pad = _CACHE[key]

    iota = np.tile(np.arange(P, dtype=np.float32), TPW)[None, :].repeat(P, 0)
    common = {
        "W1": np.asarray(inputs["W1"], np.float32),
        "W2": np.asarray(inputs["W2"], np.float32),
        "b1": np.broadcast_to(b1v[None, :], (P, b1v.size)).copy(),
        "b2": np.broadcast_to(b2v[None, :], (P, b2v.size)).copy(),
        "fcwT": np.ascontiguousarray(np.asarray(inputs["fcW"], np.float32).T),
        "fcb": np.asarray(inputs["fcb"], np.float32).reshape(1, -1),
        "iota": iota.astype(STAGE_NP),
        "ident": np.eye(P, dtype=STAGE_NP),
        "ones": np.ones((P, 1), STAGE_NP),
    }
    in_maps = []
    for c in range(n_cores):
        cd = cores[c]
        xs = np.zeros((in_ch, dsh_pad), np.float32)
        xs[:, :dsh] = x[c * dsh:(c + 1) * dsh, :].T
        in_maps.append({
            **common,
            "xT": xs,
            "gidx": cd["gidx"],
            "fidx": cd["fidx"],
            "rel": cd["rel"],
            "deg_winT": cd["deg_winT"],
            "deg_winT0": cd["deg_winT0"],
            "deg_own": cd["deg_own"],
        })
    res = bass_utils.run_bass_kernel_spmd(
        nc, in_maps, core_ids=list(range(n_cores)),
        trace=trace, trace_kwargs=trace_kwargs or {})
    out = res.results[0]["out"].reshape(2).astype(np.float32)
    return out, res


def kernel(**inputs) -> np.ndarray:
    out, _ = _run(inputs)
    return out
